# revision 14
# baseline (speedup 1.0000x reference)
"""AffinityPropagate Trainium2 kernel.

Problem: 24 iterations of a per-pixel-weighted 3x3 stencil (zero-padded)
on a [B=8, C=1, H=256, W=1216] image, weights = abs-normalized affinity
[B, 9, H, W].  Data-parallel over batch: one image per NeuronCore.

Per-core algorithm
------------------
Row i of the image maps to (partition p = i//2, slot c = i%2), so the
256 rows live on 128 partitions x 2 free-dim blocks.  With that mapping
a +-1 row shift never crosses a partition-chunk seam: it is a partition
shift by one (handled exactly by a 128x128 shifted-identity matmul,
zero-padding falls out of the missing matrix row) and/or a slot swap
(pure free-dim addressing).

Weights are normalized once, then pre-shifted so the per-iteration inner
loop is only:
  DVE : z[n]  = wsh[n] * f          (9 taps in one tensor_tensor via a
                                     stride-0 broadcast of f; fp16 -> 2x mode)
  PE  : out[c] += S_{n,c} @ z[n][shifted columns]   (PSUM fp32 accumulate)
  ACT : f' = cast(out)              (PSUM -> SBUF fp16 copy)
All column (W) shifts are plain free-dim address offsets into z's
guard-padded blocks; all row (H) shifts are the stationary matrices.
"""

import os
import sys

import numpy as np

for _p in ("/opt/trn_rl_repo", "/opt/pypackages"):
    if os.path.isdir(_p) and _p not in sys.path:
        sys.path.insert(0, _p)

B, K2, H, W = 8, 9, 256, 1216
P = 128          # partitions; row i -> (p=i//2, c=i%2)
NS = 2           # row slots per partition
GU = 2           # guard columns on each side of a z block (4B aligned)
WB = W + 2 * GU  # z/wsh block width
TILES = [(0, 406), (406, 406), (812, 404)]  # W tiling (even sizes/offsets)

# Per-bank matmul plan: out slot c' accumulates, for each tap n,
# z[n][src slot] routed through stationary matrix:
#   s = di-1 = -1: c'=0 <- (S_dn, slot 1),  c'=1 <- (I, slot 0)
#   s = 0        : c' <- (I, slot c')
#   s = +1       : c'=0 <- (I, slot 1),  c'=1 <- (S_up, slot 0)
# Stationary index: 0 = S_dn (k == m-1), 1 = I, 2 = S_up (k == m+1).
# Ordered so consecutive matmuls mostly share the stationary operand.
PLAN = {
    0: [(0, 1, 0), (1, 1, 0), (2, 1, 0),
        (3, 0, 1), (4, 0, 1), (5, 0, 1),
        (6, 1, 1), (7, 1, 1), (8, 1, 1)],
    1: [(0, 0, 1), (1, 0, 1), (2, 0, 1),
        (3, 1, 1), (4, 1, 1), (5, 1, 1),
        (6, 0, 2), (7, 0, 2), (8, 0, 2)],
}

_CACHE = {}


def _shift_mats() -> np.ndarray:
    """[3, 128, 128] fp16: S_dn (k==m-1), I, S_up (k==m+1) as lhsT[k, m]."""
    s = np.zeros((3, P, P), dtype=np.float16)
    k = np.arange(P - 1)
    s[0][k, k + 1] = 1.0      # out[m] = mov[m-1]
    s[1][np.arange(P), np.arange(P)] = 1.0
    s[2][k + 1, k] = 1.0      # out[m] = mov[m+1]
    return s


def build_program(times: int):
    import concourse.bacc as bacc
    import concourse.tile as tile
    from concourse import mybir
    from contextlib import ExitStack

    dt = mybir.dt
    nc = bacc.Bacc(trn_type="TRN2", target_bir_lowering=False, debug=False,
                   num_devices=B)

    aff = nc.dram_tensor("aff", [K2, H, W], dt.float32, kind="ExternalInput")
    feat = nc.dram_tensor("feat", [H, W], dt.float32, kind="ExternalInput")
    smat = nc.dram_tensor("smat", [3, P, P], dt.float16, kind="ExternalInput")
    out = nc.dram_tensor("out", [H, W], dt.float32, kind="ExternalOutput")

    aff_r = aff.ap().rearrange("n (p c) j -> p n c j", c=NS)
    feat_r = feat.ap().rearrange("(p c) j -> p c j", c=NS)
    out_r = out.ap().rearrange("(p c) j -> p c j", c=NS)

    with tile.TileContext(nc) as tc, ExitStack() as ctx:
        # Pool lifetimes: `persist`/`fpool` span the whole kernel; the
        # preamble pools close before `z` is allocated so the loop reuses
        # their SBUF (192 KiB/partition budget).
        persist = ctx.enter_context(tc.tile_pool(name="persist", bufs=1))
        fpool = ctx.enter_context(tc.tile_pool(name="fpool", bufs=2))
        psump = ctx.enter_context(tc.tile_pool(name="psum", bufs=8, space="PSUM"))

        smat_t = persist.tile([P, 3, P], dt.float16)
        for i in range(3):
            nc.gpsimd.dma_start(out=smat_t[:, i, :], in_=smat.ap()[i])

        wsh = persist.tile([P, K2, NS, WB], dt.float16)
        # Edge cells (shifted-out rows/cols) are never consumed downstream,
        # but zero them so every wsh read is initialized.
        nc.vector.memset(wsh, 0.0)
        fout = persist.tile([P, NS, W], dt.float32)

        f0 = fpool.tile([P, NS, W], dt.float16, tag="f")

        # DMA budget: Tile round-robins DMAs over 8 semaphore lanes and
        # every DMA after the first eight waits on its lane predecessor;
        # the DIRECT2D DMA form only carries ONE sync wait (walrus hard
        # error otherwise).  Keep the program at <= 8 total DMAs so each
        # DMA is first on its lane and only ever needs its real dep.
        with tc.tile_pool(name="pre", bufs=1) as prep, \
                tc.tile_pool(name="wstage", bufs=3) as wstagep:
            acc = prep.tile([P, NS, W], dt.float32)
            raw = prep.tile([P, K2, NS, W], dt.float32)
            feat32 = prep.tile([P, NS, W], dt.float32)

            nc.gpsimd.dma_start(out=feat32, in_=feat_r)
            nc.vector.tensor_copy(out=f0, in_=feat32)

            # ---- load (3 batched DMAs) + fused |a| accumulate (fp32) ----
            for g in range(3):
                nc.gpsimd.dma_start(out=raw[:, 3 * g:3 * g + 3],
                                    in_=aff_r[:, 3 * g:3 * g + 3])
            raws = [raw[:, n] for n in range(K2)]
            for n in range(K2):
                nc.scalar.activation(out=raws[n], in_=raws[n],
                                     func=mybir.ActivationFunctionType.Abs)
                if n == 1:
                    nc.vector.tensor_add(acc, raws[0], raws[1])
                elif n > 1:
                    nc.vector.tensor_add(acc, acc, raws[n])
            nc.vector.reciprocal(out=acc, in_=acc)
            rcp = acc

            # ---- normalize + pre-shift weights into wsh ----
            # wsh[n][i, jd] = w_n[i-s, jd-cs]  (s = di-1 rows, cs = dj-1 cols)
            # Row shift via PE shift-matmul (same stationaries as the main
            # loop, with s' = -s since here out[i] <- src[i-s]); col shift
            # via the moving operand's column offset; ACT evacuates PSUM
            # into wsh as fp16.  (DMA is avoided on purpose: DIRECT2D DMAs
            # only carry one sync wait.)
            shift_plan = {  # s -> per out-slot (stationary idx, src slot)
                1: {0: (0, 1), 1: (1, 0)},   # w[i-1]: like loop tap s'=-1
                0: {0: (1, 0), 1: (1, 1)},
                -1: {0: (1, 1), 1: (2, 0)},  # w[i+1]: like loop tap s'=+1
            }
            for n in range(K2):
                dj, di = n % 3, n // 3
                s, cs = di - 1, dj - 1
                jd0, jd1 = max(0, cs), W + min(0, cs)
                if s == 0 and cs == 0:
                    nc.vector.tensor_mul(wsh[:, n, :, GU:GU + W],
                                         raws[n], rcp)
                    continue
                ws = wstagep.tile([P, NS, W], dt.float16, tag="wstage")
                nc.vector.tensor_mul(ws, raws[n], rcp)
                for (t0, L) in TILES:
                    a0, a1 = max(jd0, t0), min(jd1, t0 + L)
                    if a0 >= a1:
                        continue
                    for c in (0, 1):
                        sidx, c_src = shift_plan[s][c]
                        ps = psump.tile([P, a1 - a0], dt.float32, tag="ps")
                        nc.tensor.matmul(out=ps, lhsT=smat_t[:, sidx, :],
                                         rhs=ws[:, c_src, a0 - cs:a1 - cs],
                                         start=True, stop=True)
                        nc.scalar.copy(out=wsh[:, n, c, GU + a0:GU + a1],
                                       in_=ps)

        loopp = ctx.enter_context(tc.tile_pool(name="loop", bufs=1))
        z = loopp.tile([P, K2, NS, WB], dt.float16)
        # z guard columns must be zero: they model the W-boundary padding.
        nc.vector.memset(z[:, :, :, 0:GU], 0.0)
        nc.vector.memset(z[:, :, :, GU + W:WB], 0.0)

        # ---- 24 stencil iterations ----
        f_cur = f0
        for k in range(times):
            last = k == times - 1
            for (t0, L) in TILES:
                fb = f_cur[:, :, t0:t0 + L].unsqueeze(1).broadcast_to(
                    [P, K2, NS, L])
                nc.vector.tensor_tensor(
                    out=z[:, :, :, GU + t0:GU + t0 + L],
                    in0=wsh[:, :, :, GU + t0:GU + t0 + L],
                    in1=fb, op=mybir.AluOpType.mult)
            f_next = None if last else fpool.tile([P, NS, W], dt.float16,
                                                  tag="f")
            for (t0, L) in TILES:
                for c in (0, 1):
                    ps = psump.tile([P, L], dt.float32, tag="ps")
                    mms = PLAN[c]
                    for mi, (n, c_src, sidx) in enumerate(mms):
                        dj = n % 3
                        m0 = GU + t0 + dj - 1
                        nc.tensor.matmul(
                            out=ps, lhsT=smat_t[:, sidx, :],
                            rhs=z[:, n, c_src, m0:m0 + L],
                            start=(mi == 0), stop=(mi == len(mms) - 1))
                    if last:
                        nc.scalar.copy(out=fout[:, c, t0:t0 + L], in_=ps)
                    else:
                        nc.scalar.copy(out=f_next[:, c, t0:t0 + L], in_=ps)
            f_cur = f_next

        nc.gpsimd.dma_start(out=out_r, in_=fout)

    nc.finalize()
    return nc


def _get_program(times: int):
    if times not in _CACHE:
        _CACHE[times] = build_program(times)
    return _CACHE[times]


def _in_maps(affinity: np.ndarray, feature: np.ndarray):
    sm = _shift_mats()
    return [{
        "aff": np.ascontiguousarray(affinity[b], dtype=np.float32),
        "feat": np.ascontiguousarray(feature[b].reshape(H, W),
                                     dtype=np.float32),
        "smat": sm,
    } for b in range(B)]


def _run(affinity, feature, times, trace=False):
    from concourse.bass_utils import run_bass_kernel_spmd

    nc = _get_program(int(times))
    res = run_bass_kernel_spmd(nc, _in_maps(affinity, feature),
                               core_ids=list(range(B)), trace=trace)
    outs = np.stack([np.asarray(res.results[b]["out"]) for b in range(B)])
    return outs.reshape(B, 1, H, W).astype(np.float32), res


def kernel(affinity, feature, times) -> np.ndarray:
    affinity = np.asarray(affinity)
    feature = np.asarray(feature)
    assert affinity.shape == (B, K2, H, W), affinity.shape
    assert feature.shape[0] == B and feature.shape[-2:] == (H, W)
    out, _ = _run(affinity, feature, int(times))
    return out


# revision 20
# speedup vs baseline: 1.5164x; 1.5164x over previous
"""AffinityPropagate Trainium2 kernel.

Problem: 24 iterations of a per-pixel-weighted 3x3 stencil (zero-padded)
on a [B=8, C=1, H=256, W=1216] image, weights = abs-normalized affinity
[B, 9, H, W].  Data-parallel over batch: one image per NeuronCore.

Per-core algorithm
------------------
Row i of the image maps to (partition p = i//2, slot c = i%2), so the
256 rows live on 128 partitions x 2 free-dim blocks.  With that mapping
a +-1 row shift never crosses a partition-chunk seam: it is a partition
shift by one (handled exactly by a 128x128 shifted-identity matmul,
zero-padding falls out of the missing matrix row) and/or a slot swap
(pure free-dim addressing).

Weights are normalized once, then pre-shifted so the per-iteration inner
loop is only:
  DVE : z[n]  = wsh[n] * f          (9 taps in one tensor_tensor via a
                                     stride-0 broadcast of f; fp16 -> 2x mode)
  PE  : out[c] += S_{n,c} @ z[n][shifted columns]   (PSUM fp32 accumulate)
  ACT : f' = cast(out)              (PSUM -> SBUF fp16 copy)
All column (W) shifts are plain free-dim address offsets into z's
guard-padded blocks; all row (H) shifts are the stationary matrices.
"""

import os
import sys

import numpy as np

for _p in ("/opt/trn_rl_repo", "/opt/pypackages"):
    if os.path.isdir(_p) and _p not in sys.path:
        sys.path.insert(0, _p)

B, K2, H, W = 8, 9, 256, 1216
P = 128          # partitions; row i -> (p=i//2, c=i%2)
NS = 2           # row slots per partition
GU = 2           # guard columns on each side of a z block (4B aligned)
WB = W + 2 * GU  # z/wsh block width
TILES = [(0, 406), (406, 406), (812, 404)]  # W tiling (even sizes/offsets)

# Per-bank matmul plan: out slot c' accumulates, for each tap n,
# z[n][src slot] routed through stationary matrix:
#   s = di-1 = -1: c'=0 <- (S_dn, slot 1),  c'=1 <- (I, slot 0)
#   s = 0        : c' <- (I, slot c')
#   s = +1       : c'=0 <- (I, slot 1),  c'=1 <- (S_up, slot 0)
# Stationary index: 0 = S_dn (k == m-1), 1 = I, 2 = S_up (k == m+1).
# Ordered so consecutive matmuls mostly share the stationary operand.
PLAN = {
    0: [(0, 1, 0), (1, 1, 0), (2, 1, 0),
        (3, 0, 1), (4, 0, 1), (5, 0, 1),
        (6, 1, 1), (7, 1, 1), (8, 1, 1)],
    1: [(0, 0, 1), (1, 0, 1), (2, 0, 1),
        (3, 1, 1), (4, 1, 1), (5, 1, 1),
        (6, 0, 2), (7, 0, 2), (8, 0, 2)],
}

_CACHE = {}


def _shift_mats() -> np.ndarray:
    """[3, 128, 128] fp16: S_dn (k==m-1), I, S_up (k==m+1) as lhsT[k, m]."""
    s = np.zeros((3, P, P), dtype=np.float16)
    k = np.arange(P - 1)
    s[0][k, k + 1] = 1.0      # out[m] = mov[m-1]
    s[1][np.arange(P), np.arange(P)] = 1.0
    s[2][k + 1, k] = 1.0      # out[m] = mov[m+1]
    return s


def build_program(times: int):
    import concourse.bacc as bacc
    import concourse.tile as tile
    from concourse import mybir
    from contextlib import ExitStack

    dt = mybir.dt
    nc = bacc.Bacc(trn_type="TRN2", target_bir_lowering=False, debug=False,
                   num_devices=B)

    aff = nc.dram_tensor("aff", [K2, H, W], dt.float32, kind="ExternalInput")
    feat = nc.dram_tensor("feat", [H, W], dt.float32, kind="ExternalInput")
    smat = nc.dram_tensor("smat", [3, P, P], dt.float16, kind="ExternalInput")
    out = nc.dram_tensor("out", [H, W], dt.float32, kind="ExternalOutput")

    aff_r = aff.ap().rearrange("n (p c) j -> p n c j", c=NS)
    feat_r = feat.ap().rearrange("(p c) j -> p c j", c=NS)
    out_r = out.ap().rearrange("(p c) j -> p c j", c=NS)

    with tile.TileContext(nc) as tc, ExitStack() as ctx:
        # Pool lifetimes: `persist`/`fpool` span the whole kernel; the
        # preamble pools close before `z` is allocated so the loop reuses
        # their SBUF (192 KiB/partition budget).
        persist = ctx.enter_context(tc.tile_pool(name="persist", bufs=1))
        fpool = ctx.enter_context(tc.tile_pool(name="fpool", bufs=2))
        psump = ctx.enter_context(tc.tile_pool(name="psum", bufs=8, space="PSUM"))

        smat_t = persist.tile([P, 3, P], dt.float16)
        for i in range(3):
            nc.gpsimd.dma_start(out=smat_t[:, i, :], in_=smat.ap()[i])

        wsh = persist.tile([P, K2, NS, WB], dt.float16)
        fout = persist.tile([P, NS, W], dt.float32)

        f0 = fpool.tile([P, NS, W], dt.float16, tag="f")

        # DMA budget: Tile round-robins DMAs over 8 semaphore lanes and
        # every DMA after the first eight waits on its lane predecessor;
        # the DIRECT2D DMA form only carries ONE sync wait (walrus hard
        # error otherwise).  Keep the program at <= 8 total DMAs so each
        # DMA is first on its lane and only ever needs its real dep.
        with tc.tile_pool(name="pre", bufs=1) as prep, \
                tc.tile_pool(name="wstage", bufs=3) as wstagep:
            acc = prep.tile([P, NS, W], dt.float32)
            raw = prep.tile([P, K2, NS, W], dt.float32)
            feat32 = prep.tile([P, NS, W], dt.float32)

            nc.gpsimd.dma_start(out=feat32, in_=feat_r)
            nc.vector.tensor_copy(out=f0, in_=feat32)

            # ---- load (3 batched DMAs) + fused |a| accumulate (fp32) ----
            for g in range(3):
                nc.gpsimd.dma_start(out=raw[:, 3 * g:3 * g + 3],
                                    in_=aff_r[:, 3 * g:3 * g + 3])
            # Zero wsh once (edge cells are never-consumed but must be
            # initialized for the sim).  On GpSimd, after the DMA issues,
            # so it rides out the load wait off the Vector engine.
            nc.gpsimd.memset(wsh, 0.0)
            raws = [raw[:, n] for n in range(K2)]
            for n in range(K2):
                nc.scalar.activation(out=raws[n], in_=raws[n],
                                     func=mybir.ActivationFunctionType.Abs)
                if n == 1:
                    nc.vector.tensor_add(acc, raws[0], raws[1])
                elif n > 1:
                    nc.vector.tensor_add(acc, acc, raws[n])
            # ~51-ULP NR reciprocal: 5x faster than the iterative-divide
            # `reciprocal` (which measured 18us here); plenty for fp16
            # weights. Inputs are sums of 9 |N(0,1)| values — no edge cases.
            nc.vector.reciprocal_approx_fast(out=acc, in_=acc)
            rcp = acc

            # ---- normalize + pre-shift weights into wsh ----
            # wsh[n][i, jd] = w_n[i-s, jd-cs]  (s = di-1 rows, cs = dj-1 cols)
            # Row shift via PE shift-matmul (same stationaries as the main
            # loop, with s' = -s since here out[i] <- src[i-s]); col shift
            # via the moving operand's column offset; ACT evacuates PSUM
            # into wsh as fp16.  (DMA is avoided on purpose: DIRECT2D DMAs
            # only carry one sync wait.)
            shift_plan = {  # s -> per out-slot (stationary idx, src slot)
                1: {0: (0, 1), 1: (1, 0)},   # w[i-1]: like loop tap s'=-1
                0: {0: (1, 0), 1: (1, 1)},
                -1: {0: (1, 1), 1: (2, 0)},  # w[i+1]: like loop tap s'=+1
            }
            for n in range(K2):
                dj, di = n % 3, n // 3
                s, cs = di - 1, dj - 1
                jd0, jd1 = max(0, cs), W + min(0, cs)
                if s == 0 and cs == 0:
                    nc.vector.tensor_mul(wsh[:, n, :, GU:GU + W],
                                         raws[n], rcp)
                    continue
                ws = wstagep.tile([P, NS, W], dt.float16, tag="wstage")
                nc.vector.tensor_mul(ws, raws[n], rcp)
                for (t0, L) in TILES:
                    a0, a1 = max(jd0, t0), min(jd1, t0 + L)
                    if a0 >= a1:
                        continue
                    for c in (0, 1):
                        sidx, c_src = shift_plan[s][c]
                        ps = psump.tile([P, a1 - a0], dt.float32, tag="ps")
                        nc.tensor.matmul(out=ps, lhsT=smat_t[:, sidx, :],
                                         rhs=ws[:, c_src, a0 - cs:a1 - cs],
                                         start=True, stop=True)
                        nc.scalar.copy(out=wsh[:, n, c, GU + a0:GU + a1],
                                       in_=ps)

        loopp = ctx.enter_context(tc.tile_pool(name="loop", bufs=1))
        z = loopp.tile([P, K2, NS, WB], dt.float16)
        # z guard columns must be zero: they model the W-boundary padding.
        nc.vector.memset(z[:, :, :, 0:GU], 0.0)
        nc.vector.memset(z[:, :, :, GU + W:WB], 0.0)

        # ---- 24 stencil iterations ----
        # Matmuls for PSUM bank (c, tile t) are split by column dependency:
        # taps with dj<=1 only read z columns from tiles <= t, so they can
        # start as soon as TT(t) lands; dj==2 taps read one column of tile
        # t+1.  This keeps PE busy throughout the DVE phase (HAM stays at
        # full clock) instead of idling until all three TTs finish.
        dj01 = {c: [e for e in PLAN[c] if e[0] % 3 <= 1] for c in (0, 1)}
        dj2 = {c: [e for e in PLAN[c] if e[0] % 3 == 2] for c in (0, 1)}

        f_cur = f0
        for k in range(times):
            last = k == times - 1
            for (t0, L) in TILES:
                fb = f_cur[:, :, t0:t0 + L].unsqueeze(1).broadcast_to(
                    [P, K2, NS, L])
                nc.vector.tensor_tensor(
                    out=z[:, :, :, GU + t0:GU + t0 + L],
                    in0=wsh[:, :, :, GU + t0:GU + t0 + L],
                    in1=fb, op=mybir.AluOpType.mult)
            f_next = None if last else fpool.tile([P, NS, W], dt.float16,
                                                  tag="f")

            def emit_mms(ps, c, t0, L, entries, start, stop):
                for mi, (n, c_src, sidx) in enumerate(entries):
                    dj = n % 3
                    m0 = GU + t0 + dj - 1
                    nc.tensor.matmul(
                        out=ps, lhsT=smat_t[:, sidx, :],
                        rhs=z[:, n, c_src, m0:m0 + L],
                        start=start and mi == 0,
                        stop=stop and mi == len(entries) - 1,
                        skip_group_check=True)

            pst = {}
            for ti, (t0, L) in enumerate(TILES):
                for c in (0, 1):
                    pst[(c, ti)] = psump.tile([P, L], dt.float32, tag="ps",
                                              name=f"ps_{c}_{ti}")
            # Phase ti: finish banks of tile ti-1 (dj2 taps + evacuate),
            # then open banks of tile ti (dj01 taps).
            for ti, (t0, L) in enumerate(TILES):
                if ti > 0:
                    p0, pl = TILES[ti - 1]
                    for c in (0, 1):
                        emit_mms(pst[(c, ti - 1)], c, p0, pl, dj2[c],
                                 start=False, stop=True)
                        dstf = fout if last else f_next
                        nc.scalar.copy(out=dstf[:, c, p0:p0 + pl],
                                       in_=pst[(c, ti - 1)])
                for c in (0, 1):
                    emit_mms(pst[(c, ti)], c, t0, L, dj01[c],
                             start=True, stop=False)
            t0, L = TILES[-1]
            for c in (0, 1):
                emit_mms(pst[(c, 2)], c, t0, L, dj2[c], start=False,
                         stop=True)
                dstf = fout if last else f_next
                nc.scalar.copy(out=dstf[:, c, t0:t0 + L], in_=pst[(c, 2)])
            f_cur = f_next

        nc.gpsimd.dma_start(out=out_r, in_=fout)

    nc.finalize()
    return nc


def _get_program(times: int):
    if times not in _CACHE:
        _CACHE[times] = build_program(times)
    return _CACHE[times]


def _in_maps(affinity: np.ndarray, feature: np.ndarray):
    sm = _shift_mats()
    return [{
        "aff": np.ascontiguousarray(affinity[b], dtype=np.float32),
        "feat": np.ascontiguousarray(feature[b].reshape(H, W),
                                     dtype=np.float32),
        "smat": sm,
    } for b in range(B)]


def _run(affinity, feature, times, trace=False):
    from concourse.bass_utils import run_bass_kernel_spmd

    nc = _get_program(int(times))
    res = run_bass_kernel_spmd(nc, _in_maps(affinity, feature),
                               core_ids=list(range(B)), trace=trace)
    outs = np.stack([np.asarray(res.results[b]["out"]) for b in range(B)])
    return outs.reshape(B, 1, H, W).astype(np.float32), res


def kernel(affinity, feature, times) -> np.ndarray:
    affinity = np.asarray(affinity)
    feature = np.asarray(feature)
    assert affinity.shape == (B, K2, H, W), affinity.shape
    assert feature.shape[0] == B and feature.shape[-2:] == (H, W)
    out, _ = _run(affinity, feature, int(times))
    return out


# revision 24
# speedup vs baseline: 1.5185x; 1.0014x over previous
"""AffinityPropagate Trainium2 kernel.

Problem: 24 iterations of a per-pixel-weighted 3x3 stencil (zero-padded)
on a [B=8, C=1, H=256, W=1216] image, weights = abs-normalized affinity
[B, 9, H, W].  Data-parallel over batch: one image per NeuronCore.

Per-core algorithm
------------------
Row i of the image maps to (partition p = i//2, slot c = i%2), so the
256 rows live on 128 partitions x 2 free-dim blocks.  With that mapping
a +-1 row shift never crosses a partition-chunk seam: it is a partition
shift by one (handled exactly by a 128x128 shifted-identity matmul,
zero-padding falls out of the missing matrix row) and/or a slot swap
(pure free-dim addressing).

Weights are normalized once, then pre-shifted so the per-iteration inner
loop is only:
  DVE : z[n]  = wsh[n] * f          (9 taps in one tensor_tensor via a
                                     stride-0 broadcast of f; fp16 -> 2x mode)
  PE  : out[c] += S_{n,c} @ z[n][shifted columns]   (PSUM fp32 accumulate)
  ACT : f' = cast(out)              (PSUM -> SBUF fp16 copy)
All column (W) shifts are plain free-dim address offsets into z's
guard-padded blocks; all row (H) shifts are the stationary matrices.
"""

import os
import sys

import numpy as np

for _p in ("/opt/trn_rl_repo", "/opt/pypackages"):
    if os.path.isdir(_p) and _p not in sys.path:
        sys.path.insert(0, _p)

B, K2, H, W = 8, 9, 256, 1216
P = 128          # partitions; row i -> (p=i//2, c=i%2)
NS = 2           # row slots per partition
GU = 2           # guard columns on each side of a z block (4B aligned)
WB = W + 2 * GU  # z/wsh block width
TILES = [(0, 406), (406, 406), (812, 404)]  # W tiling (even sizes/offsets)

# Per-bank matmul plan: out slot c' accumulates, for each tap n,
# z[n][src slot] routed through stationary matrix:
#   s = di-1 = -1: c'=0 <- (S_dn, slot 1),  c'=1 <- (I, slot 0)
#   s = 0        : c' <- (I, slot c')
#   s = +1       : c'=0 <- (I, slot 1),  c'=1 <- (S_up, slot 0)
# Stationary index: 0 = S_dn (k == m-1), 1 = I, 2 = S_up (k == m+1).
# Ordered so consecutive matmuls mostly share the stationary operand.
PLAN = {
    0: [(0, 1, 0), (1, 1, 0), (2, 1, 0),
        (3, 0, 1), (4, 0, 1), (5, 0, 1),
        (6, 1, 1), (7, 1, 1), (8, 1, 1)],
    1: [(0, 0, 1), (1, 0, 1), (2, 0, 1),
        (3, 1, 1), (4, 1, 1), (5, 1, 1),
        (6, 0, 2), (7, 0, 2), (8, 0, 2)],
}

_CACHE = {}


def _shift_mats() -> np.ndarray:
    """[3, 128, 128] fp16: S_dn (k==m-1), I, S_up (k==m+1) as lhsT[k, m]."""
    s = np.zeros((3, P, P), dtype=np.float16)
    k = np.arange(P - 1)
    s[0][k, k + 1] = 1.0      # out[m] = mov[m-1]
    s[1][np.arange(P), np.arange(P)] = 1.0
    s[2][k + 1, k] = 1.0      # out[m] = mov[m+1]
    return s


def build_program(times: int):
    import concourse.bacc as bacc
    import concourse.tile as tile
    from concourse import mybir
    from contextlib import ExitStack

    dt = mybir.dt
    nc = bacc.Bacc(trn_type="TRN2", target_bir_lowering=False, debug=False,
                   num_devices=B)

    aff = nc.dram_tensor("aff", [K2, H, W], dt.float32, kind="ExternalInput")
    feat = nc.dram_tensor("feat", [H, W], dt.float32, kind="ExternalInput")
    smat = nc.dram_tensor("smat", [3, P, P], dt.float16, kind="ExternalInput")
    out = nc.dram_tensor("out", [H, W], dt.float32, kind="ExternalOutput")

    aff_r = aff.ap().rearrange("n (p c) j -> p n c j", c=NS)
    feat_r = feat.ap().rearrange("(p c) j -> p c j", c=NS)
    out_r = out.ap().rearrange("(p c) j -> p c j", c=NS)

    with tile.TileContext(nc) as tc, ExitStack() as ctx:
        # Pool lifetimes: `persist`/`fpool` span the whole kernel; the
        # preamble pools close before `z` is allocated so the loop reuses
        # their SBUF (192 KiB/partition budget).
        persist = ctx.enter_context(tc.tile_pool(name="persist", bufs=1))
        fpool = ctx.enter_context(tc.tile_pool(name="fpool", bufs=2))
        psump = ctx.enter_context(tc.tile_pool(name="psum", bufs=8, space="PSUM"))

        smat_t = persist.tile([P, 3, P], dt.float16)
        for i in range(3):
            nc.gpsimd.dma_start(out=smat_t[:, i, :], in_=smat.ap()[i])

        wsh = persist.tile([P, K2, NS, WB], dt.float16)
        fout = persist.tile([P, NS, W], dt.float32)

        f0 = fpool.tile([P, NS, W], dt.float16, tag="f")

        # DMA budget: Tile round-robins DMAs over 8 semaphore lanes and
        # every DMA after the first eight waits on its lane predecessor;
        # the DIRECT2D DMA form only carries ONE sync wait (walrus hard
        # error otherwise).  Keep the program at <= 8 total DMAs so each
        # DMA only ever needs its one real dep.
        with tc.tile_pool(name="pre", bufs=1) as prep, \
                tc.tile_pool(name="raw3", bufs=2) as raw3p, \
                tc.tile_pool(name="wstage", bufs=2) as wstagep:
            absa = prep.tile([P, K2, NS, W], dt.float16)
            acc = prep.tile([P, NS, W], dt.float32)
            rcp = prep.tile([P, NS, W], dt.float16)

            # Affinity first (it gates the whole preamble), 3 batched DMAs
            # through a 2-slot rotating buffer; feature + shift matrices
            # after (they are needed much later).
            rbufs = []
            for g in range(3):
                rb = raw3p.tile([P, 3, NS, W], dt.float32, tag="raw3",
                                name=f"raw3_{g}")
                nc.gpsimd.dma_start(out=rb, in_=aff_r[:, 3 * g:3 * g + 3])
                rbufs.append(rb)
            feat32 = raw3p.tile([P, NS, W], dt.float32, tag="raw3")
            nc.gpsimd.dma_start(out=feat32, in_=feat_r)
            # Zero wsh once (edge cells are never-consumed but must be
            # initialized for the sim).  On GpSimd, after the DMA issues,
            # so it rides out the load wait off the Vector engine.
            nc.gpsimd.memset(wsh, 0.0)

            # ---- |a| -> fp16 (per batch, on ACT) + 2x-mode fp16 sum ----
            for g in range(3):
                nc.scalar.activation(out=absa[:, 3 * g:3 * g + 3],
                                     in_=rbufs[g],
                                     func=mybir.ActivationFunctionType.Abs)
                for n in range(3 * g, 3 * g + 3):
                    if n == 1:
                        nc.vector.tensor_add(acc, absa[:, 0], absa[:, 1])
                    elif n > 1:
                        nc.vector.tensor_add(acc, acc, absa[:, n])
            nc.scalar.copy(out=f0, in_=feat32)
            # ~51-ULP NR reciprocal (5x faster than iterative-divide
            # `reciprocal`, which measured 18us); fp32 in/out, then an ACT
            # cast to fp16 so the 9 normalize muls run in 2x mode.
            nc.vector.reciprocal_approx_fast(out=acc, in_=acc)
            nc.scalar.copy(out=rcp, in_=acc)

            # ---- normalize + pre-shift weights into wsh ----
            # wsh[n][i, jd] = w_n[i-s, jd-cs]  (s = di-1 rows, cs = dj-1 cols)
            # Row shift via PE shift-matmul (same stationaries as the main
            # loop, with s' = -s since here out[i] <- src[i-s]); col shift
            # via the moving operand's column offset; ACT evacuates PSUM
            # into wsh as fp16.  (DMA is avoided on purpose: DIRECT2D DMAs
            # only carry one sync wait.)
            shift_plan = {  # s -> per out-slot (stationary idx, src slot)
                1: {0: (0, 1), 1: (1, 0)},   # w[i-1]: like loop tap s'=-1
                0: {0: (1, 0), 1: (1, 1)},
                -1: {0: (1, 1), 1: (2, 0)},  # w[i+1]: like loop tap s'=+1
            }
            for n in range(K2):
                dj, di = n % 3, n // 3
                s, cs = di - 1, dj - 1
                jd0, jd1 = max(0, cs), W + min(0, cs)
                if s == 0 and cs == 0:
                    nc.vector.tensor_mul(wsh[:, n, :, GU:GU + W],
                                         absa[:, n], rcp)
                    continue
                ws = wstagep.tile([P, NS, W], dt.float16, tag="wstage")
                nc.vector.tensor_mul(ws, absa[:, n], rcp)
                for (t0, L) in TILES:
                    a0, a1 = max(jd0, t0), min(jd1, t0 + L)
                    if a0 >= a1:
                        continue
                    for c in (0, 1):
                        sidx, c_src = shift_plan[s][c]
                        ps = psump.tile([P, a1 - a0], dt.float32, tag="ps")
                        nc.tensor.matmul(out=ps, lhsT=smat_t[:, sidx, :],
                                         rhs=ws[:, c_src, a0 - cs:a1 - cs],
                                         start=True, stop=True)
                        nc.scalar.copy(out=wsh[:, n, c, GU + a0:GU + a1],
                                       in_=ps)

        loopp = ctx.enter_context(tc.tile_pool(name="loop", bufs=1))
        z = loopp.tile([P, K2, NS, WB], dt.float16)
        # z guard columns must be zero: they model the W-boundary padding.
        nc.vector.memset(z[:, :, :, 0:GU], 0.0)
        nc.vector.memset(z[:, :, :, GU + W:WB], 0.0)

        # ---- 24 stencil iterations ----
        # Matmuls for PSUM bank (c, tile t) are split by column dependency:
        # taps with dj<=1 only read z columns from tiles <= t, so they can
        # start as soon as TT(t) lands; dj==2 taps read one column of tile
        # t+1.  This keeps PE busy throughout the DVE phase (HAM stays at
        # full clock) instead of idling until all three TTs finish.
        dj01 = {c: [e for e in PLAN[c] if e[0] % 3 <= 1] for c in (0, 1)}
        dj2 = {c: [e for e in PLAN[c] if e[0] % 3 == 2] for c in (0, 1)}

        f_cur = f0
        for k in range(times):
            last = k == times - 1
            for (t0, L) in TILES:
                fb = f_cur[:, :, t0:t0 + L].unsqueeze(1).broadcast_to(
                    [P, K2, NS, L])
                nc.vector.tensor_tensor(
                    out=z[:, :, :, GU + t0:GU + t0 + L],
                    in0=wsh[:, :, :, GU + t0:GU + t0 + L],
                    in1=fb, op=mybir.AluOpType.mult)
            f_next = None if last else fpool.tile([P, NS, W], dt.float16,
                                                  tag="f")

            def emit_mms(ps, c, t0, L, entries, start, stop):
                for mi, (n, c_src, sidx) in enumerate(entries):
                    dj = n % 3
                    m0 = GU + t0 + dj - 1
                    nc.tensor.matmul(
                        out=ps, lhsT=smat_t[:, sidx, :],
                        rhs=z[:, n, c_src, m0:m0 + L],
                        start=start and mi == 0,
                        stop=stop and mi == len(entries) - 1,
                        skip_group_check=True)

            pst = {}
            for ti, (t0, L) in enumerate(TILES):
                for c in (0, 1):
                    pst[(c, ti)] = psump.tile([P, L], dt.float32, tag="ps",
                                              name=f"ps_{c}_{ti}")
            # Phase ti: finish banks of tile ti-1 (dj2 taps + evacuate),
            # then open banks of tile ti (dj01 taps).
            for ti, (t0, L) in enumerate(TILES):
                if ti > 0:
                    p0, pl = TILES[ti - 1]
                    for c in (0, 1):
                        emit_mms(pst[(c, ti - 1)], c, p0, pl, dj2[c],
                                 start=False, stop=True)
                        dstf = fout if last else f_next
                        nc.scalar.copy(out=dstf[:, c, p0:p0 + pl],
                                       in_=pst[(c, ti - 1)])
                for c in (0, 1):
                    emit_mms(pst[(c, ti)], c, t0, L, dj01[c],
                             start=True, stop=False)
            t0, L = TILES[-1]
            for c in (0, 1):
                emit_mms(pst[(c, 2)], c, t0, L, dj2[c], start=False,
                         stop=True)
                dstf = fout if last else f_next
                nc.scalar.copy(out=dstf[:, c, t0:t0 + L], in_=pst[(c, 2)])
            f_cur = f_next

        nc.gpsimd.dma_start(out=out_r, in_=fout)

    nc.finalize()
    return nc


def _get_program(times: int):
    if times not in _CACHE:
        _CACHE[times] = build_program(times)
    return _CACHE[times]


def _in_maps(affinity: np.ndarray, feature: np.ndarray):
    sm = _shift_mats()
    return [{
        "aff": np.ascontiguousarray(affinity[b], dtype=np.float32),
        "feat": np.ascontiguousarray(feature[b].reshape(H, W),
                                     dtype=np.float32),
        "smat": sm,
    } for b in range(B)]


def _run(affinity, feature, times, trace=False):
    from concourse.bass_utils import run_bass_kernel_spmd

    nc = _get_program(int(times))
    res = run_bass_kernel_spmd(nc, _in_maps(affinity, feature),
                               core_ids=list(range(B)), trace=trace)
    outs = np.stack([np.asarray(res.results[b]["out"]) for b in range(B)])
    return outs.reshape(B, 1, H, W).astype(np.float32), res


def kernel(affinity, feature, times) -> np.ndarray:
    affinity = np.asarray(affinity)
    feature = np.asarray(feature)
    assert affinity.shape == (B, K2, H, W), affinity.shape
    assert feature.shape[0] == B and feature.shape[-2:] == (H, W)
    out, _ = _run(affinity, feature, int(times))
    return out


# revision 28
# speedup vs baseline: 1.5647x; 1.0304x over previous
"""AffinityPropagate Trainium2 kernel.

Problem: 24 iterations of a per-pixel-weighted 3x3 stencil (zero-padded)
on a [B=8, C=1, H=256, W=1216] image, weights = abs-normalized affinity
[B, 9, H, W].  Data-parallel over batch: one image per NeuronCore.

Per-core algorithm
------------------
Row i of the image maps to (partition p = i//2, slot c = i%2), so the
256 rows live on 128 partitions x 2 free-dim blocks.  With that mapping
a +-1 row shift never crosses a partition-chunk seam: it is a partition
shift by one (handled exactly by a 128x128 shifted-identity matmul,
zero-padding falls out of the missing matrix row) and/or a slot swap
(pure free-dim addressing).

Weights are normalized once, then pre-shifted so the per-iteration inner
loop is only:
  DVE : z[n]  = wsh[n] * f          (9 taps in one tensor_tensor via a
                                     stride-0 broadcast of f; fp16 -> 2x mode)
  PE  : out[c] += S_{n,c} @ z[n][shifted columns]   (PSUM fp32 accumulate)
  ACT : f' = cast(out)              (PSUM -> SBUF fp16 copy)
All column (W) shifts are plain free-dim address offsets into z's
guard-padded blocks; all row (H) shifts are the stationary matrices.
"""

import os
import sys

import numpy as np

for _p in ("/opt/trn_rl_repo", "/opt/pypackages"):
    if os.path.isdir(_p) and _p not in sys.path:
        sys.path.insert(0, _p)

B, K2, H, W = 8, 9, 256, 1216
P = 128          # partitions; row i -> (p=i//2, c=i%2)
NS = 2           # row slots per partition
GU = 2           # guard columns on each side of a z block (4B aligned)
WB = W + 2 * GU  # z/wsh block width
TILES = [(0, 406), (406, 406), (812, 404)]  # W tiling (even sizes/offsets)

# Per-bank matmul plan: out slot c' accumulates, for each tap n,
# z[n][src slot] routed through stationary matrix:
#   s = di-1 = -1: c'=0 <- (S_dn, slot 1),  c'=1 <- (I, slot 0)
#   s = 0        : c' <- (I, slot c')
#   s = +1       : c'=0 <- (I, slot 1),  c'=1 <- (S_up, slot 0)
# Stationary index: 0 = S_dn (k == m-1), 1 = I, 2 = S_up (k == m+1).
# Ordered so consecutive matmuls mostly share the stationary operand.
PLAN = {
    0: [(0, 1, 0), (1, 1, 0), (2, 1, 0),
        (3, 0, 1), (4, 0, 1), (5, 0, 1),
        (6, 1, 1), (7, 1, 1), (8, 1, 1)],
    1: [(0, 0, 1), (1, 0, 1), (2, 0, 1),
        (3, 1, 1), (4, 1, 1), (5, 1, 1),
        (6, 0, 2), (7, 0, 2), (8, 0, 2)],
}

_CACHE = {}


def _shift_mats() -> np.ndarray:
    """[3, 128, 128] fp16: S_dn (k==m-1), I, S_up (k==m+1) as lhsT[k, m]."""
    s = np.zeros((3, P, P), dtype=np.float16)
    k = np.arange(P - 1)
    s[0][k, k + 1] = 1.0      # out[m] = mov[m-1]
    s[1][np.arange(P), np.arange(P)] = 1.0
    s[2][k + 1, k] = 1.0      # out[m] = mov[m+1]
    return s


def build_program(times: int):
    import concourse.bacc as bacc
    import concourse.tile as tile
    from concourse import mybir
    from contextlib import ExitStack

    dt = mybir.dt
    nc = bacc.Bacc(trn_type="TRN2", target_bir_lowering=False, debug=False,
                   num_devices=B)

    aff = nc.dram_tensor("aff", [K2, H, W], dt.float32, kind="ExternalInput")
    feat = nc.dram_tensor("feat", [H, W], dt.float32, kind="ExternalInput")
    smat = nc.dram_tensor("smat", [3, P, P], dt.float16, kind="ExternalInput")
    out = nc.dram_tensor("out", [H, W], dt.float32, kind="ExternalOutput")

    aff_r = aff.ap().rearrange("n (p c) j -> p n c j", c=NS)
    feat_r = feat.ap().rearrange("(p c) j -> p c j", c=NS)
    out_r = out.ap().rearrange("(p c) j -> p c j", c=NS)

    with tile.TileContext(nc) as tc, ExitStack() as ctx:
        # Pool lifetimes: `persist`/`fpool` span the whole kernel; the
        # preamble pools close before `z` is allocated so the loop reuses
        # their SBUF (192 KiB/partition budget).
        persist = ctx.enter_context(tc.tile_pool(name="persist", bufs=1))
        fpool = ctx.enter_context(tc.tile_pool(name="fpool", bufs=2))

        smat_t = persist.tile([P, 3, P], dt.float16)
        wsh = persist.tile([P, K2, NS, WB], dt.float16)
        fout = persist.tile([P, NS, W], dt.float32)

        f0 = fpool.tile([P, NS, W], dt.float16, tag="f")

        # DMA budget: Tile round-robins DMAs over 8 semaphore lanes and
        # every DMA after the first eight waits on its lane predecessor;
        # the DIRECT2D DMA form only carries ONE sync wait (walrus hard
        # error otherwise).  Keep the program at <= 8 total DMAs so each
        # DMA only ever needs its one real dep.
        with tc.tile_pool(name="pre", bufs=1) as prep, \
                tc.tile_pool(name="raw3", bufs=3) as raw3p, \
                tc.tile_pool(name="wstage", bufs=2) as wstagep, \
                tc.tile_pool(name="prepsum", bufs=2, space="PSUM") as prepsum:
            absa = prep.tile([P, K2, NS, W], dt.float16)
            acc = prep.tile([P, NS, W], dt.float32)
            rcp = prep.tile([P, NS, W], dt.float16)

            # Affinity first (it gates the whole preamble): five DMAs of
            # (2,2,2,2,1) maps through three rotating slots.  Slot reuse
            # starts at the 4th DMA, whose WAR wait (on the slot's abs
            # reads) resolves well before HBM gets to it, so the loads
            # stream back-to-back at HBM rate.  Feature + shift matrices
            # go last (needed much later).
            batches = [(0, 2), (2, 2), (4, 2), (6, 2), (8, 1)]
            rbufs = []
            for g, (n0, cnt) in enumerate(batches):
                rb = raw3p.tile([P, 2, NS, W], dt.float32, tag="raw3",
                                name=f"raw3_{g}")
                nc.gpsimd.dma_start(out=rb[:, 0:cnt],
                                    in_=aff_r[:, n0:n0 + cnt])
                rbufs.append(rb)
            feat32 = raw3p.tile([P, NS, W], dt.float32, tag="raw3")
            nc.gpsimd.dma_start(out=feat32, in_=feat_r)
            nc.gpsimd.dma_start(out=smat_t, in_=smat.ap().transpose([1, 0, 2]))
            # Zero wsh once (edge cells are never-consumed but must be
            # initialized for the sim).  On GpSimd, after the DMA issues,
            # so it rides out the load wait off the Vector engine.
            nc.gpsimd.memset(wsh, 0.0)

            # ---- |a| -> fp16 (per map, on ACT) + fp32-accumulated sum ----
            for g, (n0, cnt) in enumerate(batches):
                for q in range(cnt):
                    n = n0 + q
                    nc.scalar.activation(out=absa[:, n], in_=rbufs[g][:, q],
                                         func=mybir.ActivationFunctionType.Abs)
                    if n == 1:
                        nc.vector.tensor_add(acc, absa[:, 0], absa[:, 1])
                    elif n > 1:
                        nc.vector.tensor_add(acc, acc, absa[:, n])
            nc.scalar.copy(out=f0, in_=feat32)
            # ~51-ULP NR reciprocal (5x faster than iterative-divide
            # `reciprocal`, which measured 18us); fp32 in/out, then an ACT
            # cast to fp16 so the 9 normalize muls run in 2x mode.
            nc.vector.reciprocal_approx_fast(out=acc, in_=acc)
            nc.scalar.copy(out=rcp, in_=acc)

            # ---- normalize + pre-shift weights into wsh ----
            # wsh[n][i, jd] = w_n[i-s, jd-cs]  (s = di-1 rows, cs = dj-1 cols)
            # Row shift via PE shift-matmul (same stationaries as the main
            # loop, with s' = -s since here out[i] <- src[i-s]); col shift
            # via the moving operand's column offset; ACT evacuates PSUM
            # into wsh as fp16.  (DMA is avoided on purpose: DIRECT2D DMAs
            # only carry one sync wait.)
            shift_plan = {  # s -> per out-slot (stationary idx, src slot)
                1: {0: (0, 1), 1: (1, 0)},   # w[i-1]: like loop tap s'=-1
                0: {0: (1, 0), 1: (1, 1)},
                -1: {0: (1, 1), 1: (2, 0)},  # w[i+1]: like loop tap s'=+1
            }
            for n in range(K2):
                dj, di = n % 3, n // 3
                s, cs = di - 1, dj - 1
                jd0, jd1 = max(0, cs), W + min(0, cs)
                if s == 0 and cs == 0:
                    nc.vector.tensor_mul(wsh[:, n, :, GU:GU + W],
                                         absa[:, n], rcp)
                    continue
                ws = wstagep.tile([P, NS, W], dt.float16, tag="wstage")
                nc.vector.tensor_mul(ws, absa[:, n], rcp)
                for c in (0, 1):
                    sidx, c_src = shift_plan[s][c]
                    # 3-bank PSUM tile; matmuls land at natural column
                    # offsets (each within one 512-col bank) so one ACT
                    # copy evacuates the whole row afterwards.
                    psb = prepsum.tile([P, 1536], dt.float32, tag="pps",
                                       name=f"pps_{n}_{c}")
                    for t0 in (0, 512, 1024):
                        a0, a1 = max(jd0, t0), min(jd1, t0 + 512)
                        if a0 >= a1:
                            continue
                        nc.tensor.matmul(out=psb[:, a0:a1],
                                         lhsT=smat_t[:, sidx, :],
                                         rhs=ws[:, c_src, a0 - cs:a1 - cs],
                                         start=True, stop=True)
                    nc.scalar.copy(out=wsh[:, n, c, GU + jd0:GU + jd1],
                                   in_=psb[:, jd0:jd1])

        psump = ctx.enter_context(tc.tile_pool(name="psum", bufs=8,
                                               space="PSUM"))
        loopp = ctx.enter_context(tc.tile_pool(name="loop", bufs=1))
        z = loopp.tile([P, K2, NS, WB], dt.float16)
        # z guard columns must be zero: they model the W-boundary padding.
        nc.vector.memset(z[:, :, :, 0:GU], 0.0)
        nc.vector.memset(z[:, :, :, GU + W:WB], 0.0)

        # ---- 24 stencil iterations ----
        # Matmuls for PSUM bank (c, tile t) are split by column dependency:
        # taps with dj<=1 only read z columns from tiles <= t, so they can
        # start as soon as TT(t) lands; dj==2 taps read one column of tile
        # t+1.  This keeps PE busy throughout the DVE phase (HAM stays at
        # full clock) instead of idling until all three TTs finish.
        dj01 = {c: [e for e in PLAN[c] if e[0] % 3 <= 1] for c in (0, 1)}
        dj2 = {c: [e for e in PLAN[c] if e[0] % 3 == 2] for c in (0, 1)}

        f_cur = f0
        for k in range(times):
            last = k == times - 1
            for (t0, L) in TILES:
                fb = f_cur[:, :, t0:t0 + L].unsqueeze(1).broadcast_to(
                    [P, K2, NS, L])
                nc.vector.tensor_tensor(
                    out=z[:, :, :, GU + t0:GU + t0 + L],
                    in0=wsh[:, :, :, GU + t0:GU + t0 + L],
                    in1=fb, op=mybir.AluOpType.mult)
            f_next = None if last else fpool.tile([P, NS, W], dt.float16,
                                                  tag="f")

            def emit_mms(ps, c, t0, L, entries, start, stop):
                for mi, (n, c_src, sidx) in enumerate(entries):
                    dj = n % 3
                    m0 = GU + t0 + dj - 1
                    nc.tensor.matmul(
                        out=ps, lhsT=smat_t[:, sidx, :],
                        rhs=z[:, n, c_src, m0:m0 + L],
                        start=start and mi == 0,
                        stop=stop and mi == len(entries) - 1,
                        skip_group_check=True)

            pst = {}
            for ti, (t0, L) in enumerate(TILES):
                for c in (0, 1):
                    pst[(c, ti)] = psump.tile([P, L], dt.float32, tag="ps",
                                              name=f"ps_{c}_{ti}")
            # Phase ti: finish banks of tile ti-1 (dj2 taps + evacuate),
            # then open banks of tile ti (dj01 taps).
            for ti, (t0, L) in enumerate(TILES):
                if ti > 0:
                    p0, pl = TILES[ti - 1]
                    for c in (0, 1):
                        emit_mms(pst[(c, ti - 1)], c, p0, pl, dj2[c],
                                 start=False, stop=True)
                        dstf = fout if last else f_next
                        nc.scalar.copy(out=dstf[:, c, p0:p0 + pl],
                                       in_=pst[(c, ti - 1)])
                for c in (0, 1):
                    emit_mms(pst[(c, ti)], c, t0, L, dj01[c],
                             start=True, stop=False)
            t0, L = TILES[-1]
            for c in (0, 1):
                emit_mms(pst[(c, 2)], c, t0, L, dj2[c], start=False,
                         stop=True)
                dstf = fout if last else f_next
                nc.scalar.copy(out=dstf[:, c, t0:t0 + L], in_=pst[(c, 2)])
            f_cur = f_next

        nc.gpsimd.dma_start(out=out_r, in_=fout)

    nc.finalize()
    return nc


def _get_program(times: int):
    if times not in _CACHE:
        _CACHE[times] = build_program(times)
    return _CACHE[times]


def _in_maps(affinity: np.ndarray, feature: np.ndarray):
    sm = _shift_mats()
    return [{
        "aff": np.ascontiguousarray(affinity[b], dtype=np.float32),
        "feat": np.ascontiguousarray(feature[b].reshape(H, W),
                                     dtype=np.float32),
        "smat": sm,
    } for b in range(B)]


def _run(affinity, feature, times, trace=False):
    from concourse.bass_utils import run_bass_kernel_spmd

    nc = _get_program(int(times))
    res = run_bass_kernel_spmd(nc, _in_maps(affinity, feature),
                               core_ids=list(range(B)), trace=trace)
    outs = np.stack([np.asarray(res.results[b]["out"]) for b in range(B)])
    return outs.reshape(B, 1, H, W).astype(np.float32), res


def kernel(affinity, feature, times) -> np.ndarray:
    affinity = np.asarray(affinity)
    feature = np.asarray(feature)
    assert affinity.shape == (B, K2, H, W), affinity.shape
    assert feature.shape[0] == B and feature.shape[-2:] == (H, W)
    out, _ = _run(affinity, feature, int(times))
    return out
